# revision 1
# baseline (speedup 1.0000x reference)
"""Trainium2 Bass kernel for the Aligner2 problem.

Computes, for each batch b:
  k = LReLU(conv3(LReLU(conv3(keys))))        # [256, 520] (pad 3, kernel 3 twice)
  q = LReLU(conv7(LReLU(conv7(LReLU(conv7(queries))))))  # [256, 2048]
  raw[t,s]  = sum_c q[c,t] k[c,s] - 0.5*k2[s]            # PE, fp32r + bf16 rank-1
  l = 2*TEMP*raw   (the -TEMP*q2 term cancels in log_softmax)
  logp = l - logsumexp_s(l);  attn = exp(logp)

Sharded data-parallel over batch across 8 NeuronCores (4 batches/core).
"""
import numpy as np

import concourse.bass as bass
import concourse.bacc as bacc
import concourse.tile as tile
from concourse import mybir
from concourse.bass_utils import run_bass_kernel_spmd

F32 = mybir.dt.float32
F32R = mybir.dt.float32r
BF16 = mybir.dt.bfloat16
AF = mybir.ActivationFunctionType

SLOPE = 0.3
TEMPERATURE = 0.0005
SC = 2.0 * TEMPERATURE  # scale applied to the raw PE scores

BPC = 4          # batches per core
N_CORES = 8
D_DEC, TQ = 80, 2048
D_ENC, TK = 512, 512
DH = 256
TK1 = TK + 4     # 516 after key conv1 (kernel 3, pad 3)
TK2 = TK + 8     # 520 after key conv2
HT1 = TK1 // 2   # 258
HT2 = TK2 // 2   # 260

# act_info.json set containing Prelu, Exp, Ln, Copy, Identity together
ACT_SET_ALL = 6  # natural_log_exp_and_others

DT_MM = BF16     # matmul operand dtype: F32R or BF16


def _r32(ap):
    # view a DT_MM tile as plain f32 for DVE/DMA readers (no-op for bf16)
    return ap.bitcast(F32) if DT_MM == F32R else ap


def build_program(repeat=1):
    nc = bacc.Bacc("TRN2", target_bir_lowering=False)

    # ---------------- DRAM I/O ----------------
    q_in = nc.dram_tensor("queries", [BPC, D_DEC, TQ + 6], DT_MM, kind="ExternalInput")
    k_in = nc.dram_tensor("keys", [BPC, D_ENC, TK + 6], DT_MM, kind="ExternalInput")
    z_in = nc.dram_tensor("zpad", [128, 4], DT_MM, kind="ExternalInput")
    kw1t_d = nc.dram_tensor("kw1t", [4, 128, 3, DH], DT_MM, kind="ExternalInput")
    kw2t_d = nc.dram_tensor("kw2t", [2, 128, 3, DH], DT_MM, kind="ExternalInput")
    qw1t_d = nc.dram_tensor("qw1t", [D_DEC, 7, DH], DT_MM, kind="ExternalInput")
    qw2t_d = nc.dram_tensor("qw2t", [2, 128, 7, DH], DT_MM, kind="ExternalInput")
    qw3t_d = nc.dram_tensor("qw3t", [2, 128, 7, DH], DT_MM, kind="ExternalInput")
    kb1_d = nc.dram_tensor("kb1c", [2, 128, 1], F32, kind="ExternalInput")
    kb2_d = nc.dram_tensor("kb2c", [2, 128, 1], F32, kind="ExternalInput")
    qb1_d = nc.dram_tensor("qb1c", [2, 128, 1], F32, kind="ExternalInput")
    qb2_d = nc.dram_tensor("qb2c", [2, 128, 1], F32, kind="ExternalInput")
    qb3_d = nc.dram_tensor("qb3c", [2, 128, 1], F32, kind="ExternalInput")
    attn_out = nc.dram_tensor("attn_out", [BPC, TQ, TK2], F32, kind="ExternalOutput")
    logp_out = nc.dram_tensor("logp_out", [BPC, TQ, TK2], F32, kind="ExternalOutput")

    with tile.TileContext(nc) as tc:
        for _ in range(repeat):
            _emit(nc, tc, q_in, k_in, z_in, kw1t_d, kw2t_d, qw1t_d, qw2t_d,
                  qw3t_d, kb1_d, kb2_d, qb1_d, qb2_d, qb3_d, attn_out, logp_out)
    nc.compile()
    return nc


def _emit(nc, tc, q_in, k_in, z_in, kw1t_d, kw2t_d, qw1t_d, qw2t_d, qw3t_d,
          kb1_d, kb2_d, qb1_d, qb2_d, qb3_d, attn_out, logp_out, mode="full"):
    from contextlib import ExitStack
    ctx = ExitStack()
    with ctx:
        singles = ctx.enter_context(tc.tile_pool(name="singles", bufs=1))
        p_in = ctx.enter_context(tc.tile_pool(name="p_in", bufs=2))
        p_mid = ctx.enter_context(tc.tile_pool(name="p_mid", bufs=1))
        p_soft = ctx.enter_context(tc.tile_pool(name="p_soft", bufs=3))
        p_small = ctx.enter_context(tc.tile_pool(name="p_small", bufs=8))
        pp_conv = ctx.enter_context(
            tc.tile_pool(name="pp_conv", bufs=2, space="PSUM"))
        pp_score = ctx.enter_context(
            tc.tile_pool(name="pp_score", bufs=2, space="PSUM"))

        # Pin the ACT LUT set that serves Prelu/Exp/Ln/Copy together.
        nc.scalar.add_instruction(mybir.InstLoadActFuncSet(
            name=nc.get_next_instruction_name(), ins=[], outs=[],
            act_func_set_id=ACT_SET_ALL))

        # ---------------- weights into SBUF (once) ----------------
        w_kw1 = singles.tile([128, 4, 3, DH], DT_MM)
        for c in range(4):
            nc.sync.dma_start(out=w_kw1[:, c], in_=kw1t_d[c])
        w_kw2 = singles.tile([128, 2, 3, DH], DT_MM)
        for c in range(2):
            nc.sync.dma_start(out=w_kw2[:, c], in_=kw2t_d[c])
        w_qw1 = singles.tile([128, 7, DH], DT_MM)
        nc.sync.dma_start(out=w_qw1[:D_DEC], in_=qw1t_d[:])
        w_qw2 = singles.tile([128, 2, 7, DH], DT_MM)
        for c in range(2):
            nc.sync.dma_start(out=w_qw2[:, c], in_=qw2t_d[c])
        w_qw3 = singles.tile([128, 2, 7, DH], DT_MM)
        for c in range(2):
            nc.sync.dma_start(out=w_qw3[:, c], in_=qw3t_d[c])

        b_k1 = singles.tile([128, 2], F32)
        b_k2 = singles.tile([128, 2], F32)
        b_q1 = singles.tile([128, 2], F32)
        b_q2 = singles.tile([128, 2], F32)
        b_q3 = singles.tile([128, 2], F32)
        for sb_t, dr in ((b_k1, kb1_d), (b_k2, kb2_d), (b_q1, qb1_d),
                         (b_q2, qb2_d), (b_q3, qb3_d)):
            for h in range(2):
                nc.sync.dma_start(out=sb_t[:, h:h + 1], in_=dr[h])

        ones_row = singles.tile([1, 128], BF16)   # lhsT for k2 broadcast
        nc.vector.memset(ones_row, 1.0)
        ones_col = singles.tile([128, 1], BF16)   # lhsT for k2 reduction
        nc.vector.memset(ones_col, 1.0)

        # persistent padded intermediates; margins zeroed once via DMA
        k1pad = singles.tile([128, 2, TK1 + 6], DT_MM)
        q1pad = singles.tile([128, 2, TQ + 6], DT_MM)
        q2pad = singles.tile([128, 2, TQ + 6], DT_MM)
        for h in range(2):
            nc.sync.dma_start(out=k1pad[:, h, 0:3], in_=z_in[:, 0:3])
            nc.sync.dma_start(out=k1pad[:, h, TK1 + 3:TK1 + 6], in_=z_in[:, 0:3])
            nc.sync.dma_start(out=q1pad[:, h, 0:3], in_=z_in[:, 0:3])
            nc.sync.dma_start(out=q1pad[:, h, TQ + 3:TQ + 6], in_=z_in[:, 0:3])
            nc.sync.dma_start(out=q2pad[:, h, 0:3], in_=z_in[:, 0:3])
            nc.sync.dma_start(out=q2pad[:, h, TQ + 3:TQ + 6], in_=z_in[:, 0:3])

        for b in range(BPC):
            _emit_batch(nc, tc, b,
                        q_in, k_in, attn_out, logp_out,
                        w_kw1, w_kw2, w_qw1, w_qw2, w_qw3,
                        b_k1, b_k2, b_q1, b_q2, b_q3,
                        ones_row, ones_col, k1pad, q1pad, q2pad,
                        p_in, p_mid, p_soft, p_small, pp_conv, pp_score,
                        mode=mode)


def _emit_batch(nc, tc, b, q_in, k_in, attn_out, logp_out,
                w_kw1, w_kw2, w_qw1, w_qw2, w_qw3,
                b_k1, b_k2, b_q1, b_q2, b_q3, ones_row, ones_col,
                k1pad, q1pad, q2pad,
                p_in, p_mid, p_soft, p_small, pp_conv, pp_score, mode="full"):
    mm = nc.tensor.matmul
    act = nc.scalar.activation

    bi = b if k_in.shape[0] > 1 else 0

    # ---------------- keys path ----------------
    kpad = p_in.tile([128, 4, TK + 6], DT_MM, tag="kpad")
    for c in range(4):
        nc.sync.dma_start(out=kpad[:, c, :],
                          in_=k_in[bi, 128 * c:128 * (c + 1), :])

    # key conv1: Cin=512, K=3, out [256, 516] -> k1pad with 3-margins
    for h in range(2):
        ps = pp_conv.tile([128, 2, 512], F32, tag="conv")
        for c in range(4):
            for j in range(3):
                for th in range(2):
                    mm(ps[:, th, :HT1],
                       w_kw1[:, c, j, 128 * h:128 * (h + 1)],
                       kpad[:, c, HT1 * th + j:HT1 * th + j + HT1],
                       start=(c == 0 and j == 0), stop=(c == 3 and j == 2))
        act(k1pad[:, h, 3:3 + TK1], ps[:, :, :HT1],
            AF.Prelu, bias=b_k1[:, h:h + 1], scale=1.0, alpha=SLOPE)

    # key conv2: Cin=256, K=3, out [256, 520]
    ksb = p_mid.tile([128, 2, TK2], DT_MM, tag="ksb")
    for h in range(2):
        ps = pp_conv.tile([128, 2, 512], F32, tag="conv")
        for c in range(2):
            for j in range(3):
                for sh in range(2):
                    mm(ps[:, sh, :HT2],
                       w_kw2[:, c, j, 128 * h:128 * (h + 1)],
                       k1pad[:, c, HT2 * sh + j:HT2 * sh + j + HT2],
                       start=(c == 0 and j == 0), stop=(c == 1 and j == 2))
        act(ksb[:, h, :], ps[:, :, :HT2],
            AF.Prelu, bias=b_k2[:, h:h + 1], scale=1.0, alpha=SLOPE)

    # k2[s] = sum_c k[c,s]^2  -> k2row = -0.5*k2 (bf16, one partition)
    ksq = p_mid.tile([128, 2, TK2], BF16, tag="ksq")
    nc.vector.tensor_mul(ksq[:, :, :], _r32(ksb[:, :, :]),
                         _r32(ksb[:, :, :]))
    k2row = p_mid.tile([1, TK2], BF16, tag="k2row")
    ps2 = pp_score.tile([1, 2, 512], F32, tag="score")
    for sh in range(2):
        for c in range(2):
            mm(ps2[:, sh, :HT2], ones_col[:, :],
               ksq[:, c, HT2 * sh:HT2 * sh + HT2],
               start=(c == 0), stop=(c == 1))
    act(k2row[:, :].rearrange("p (a b) -> p a b", a=2), ps2[:, :, :HT2],
        AF.Copy, bias=0.0, scale=-0.5)

    # ---------------- queries path ----------------
    qpad = p_in.tile([128, TQ + 6], DT_MM, tag="qpad")
    nc.sync.dma_start(out=qpad[:D_DEC, :], in_=q_in[bi])

    for h in range(2):
        for g in range(2):  # pairs of 512-wide t-chunks
            ps = pp_conv.tile([128, 2, 512], F32, tag="conv")
            for j in range(7):
                for i in range(2):
                    t4 = 2 * g + i
                    mm(ps[:, i, :],
                       w_qw1[:D_DEC, j, 128 * h:128 * (h + 1)],
                       qpad[:D_DEC, 512 * t4 + j:512 * t4 + j + 512],
                       start=(j == 0), stop=(j == 6))
            act(q1pad[:, h, 3 + 1024 * g:3 + 1024 * (g + 1)], ps[:, :, :],
                AF.Prelu, bias=b_q1[:, h:h + 1], scale=1.0, alpha=SLOPE)

    for h in range(2):
        for g in range(2):
            ps = pp_conv.tile([128, 2, 512], F32, tag="conv")
            for c in range(2):
                for j in range(7):
                    for i in range(2):
                        t4 = 2 * g + i
                        mm(ps[:, i, :],
                           w_qw2[:, c, j, 128 * h:128 * (h + 1)],
                           q1pad[:, c, 512 * t4 + j:512 * t4 + j + 512],
                           start=(c == 0 and j == 0), stop=(c == 1 and j == 6))
            act(q2pad[:, h, 3 + 1024 * g:3 + 1024 * (g + 1)], ps[:, :, :],
                AF.Prelu, bias=b_q2[:, h:h + 1], scale=1.0, alpha=SLOPE)

    q3 = p_mid.tile([128, 2, TQ], DT_MM, tag="q3")
    for h in range(2):
        for g in range(2):
            ps = pp_conv.tile([128, 2, 512], F32, tag="conv")
            for c in range(2):
                for j in range(7):
                    for i in range(2):
                        t4 = 2 * g + i
                        mm(ps[:, i, :],
                           w_qw3[:, c, j, 128 * h:128 * (h + 1)],
                           q2pad[:, c, 512 * t4 + j:512 * t4 + j + 512],
                           start=(c == 0 and j == 0), stop=(c == 1 and j == 6))
            act(q3[:, h, 1024 * g:1024 * (g + 1)], ps[:, :, :],
                AF.Prelu, bias=b_q3[:, h:h + 1], scale=1.0, alpha=SLOPE)

    # ---------------- scores + softmax ----------------
    if mode == "noscore":
        nc.gpsimd.dma_start(out=attn_out[b, 0:128, :],
                            in_=_r32(q3[:, 0, 0:TK2]))
        return
    for t in range(TQ // 128):
        sp = pp_score.tile([128, 2, 512], F32, tag="score", name=f"sp{b}_{t}")
        for c in range(2):
            for sh in range(2):
                mm(sp[:, sh, :HT2],
                   q3[:, c, 128 * t:128 * (t + 1)],
                   ksb[:, c, HT2 * sh:HT2 * sh + HT2],
                   start=(c == 0), stop=False)
        for sh in range(2):
            mm(sp[:, sh, :HT2], ones_row[:, :], k2row[:, HT2 * sh:HT2 * sh + HT2],
               start=False, stop=True)

        if mode == "nosoftmax":
            raw_sb = p_soft.tile([128, TK2], F32, tag="esb")
            act(raw_sb[:, :].rearrange("p (a b) -> p a b", a=2), sp[:, :, :HT2],
                AF.Copy, bias=0.0, scale=SC)
            nc.sync.dma_start(out=attn_out[b, 128 * t:128 * (t + 1), :],
                              in_=raw_sb)
            continue
        esb = p_soft.tile([128, TK2], F32, tag="esb")
        z = p_small.tile([128, 1], F32, tag="z")
        # exp is the ONLY psum reader -> the score bank frees after one hop
        act(esb[:, :].rearrange("p (a b) -> p a b", a=2), sp[:, :, :HT2],
            AF.Exp, bias=0.0, scale=SC, accum_out=z)
        rz = p_small.tile([128, 1], F32, tag="rz")
        nc.vector.reciprocal(rz, z)
        attn_sb = p_soft.tile([128, TK2], F32, tag="attn")
        nc.vector.tensor_scalar_mul(attn_sb, esb, rz)
        logp_sb = p_soft.tile([128, TK2], F32, tag="logp")
        act(logp_sb, attn_sb, AF.Ln)

        nc.sync.dma_start(out=attn_out[b, 128 * t:128 * (t + 1), :], in_=attn_sb)
        nc.sync.dma_start(out=logp_out[b, 128 * t:128 * (t + 1), :], in_=logp_sb)


def build_timing_program(repeat=1, mode="full"):
    """Same compute, but single-batch external inputs reused for all batches,
    outputs to Internal DRAM scratch + tiny canary output: removes the
    hundreds-of-MB per-call transfer so wall-clock deltas measure exec."""
    nc = bacc.Bacc("TRN2", target_bir_lowering=False)
    q_in = nc.dram_tensor("queries", [1, D_DEC, TQ + 6], DT_MM, kind="ExternalInput")
    k_in = nc.dram_tensor("keys", [1, D_ENC, TK + 6], DT_MM, kind="ExternalInput")
    z_in = nc.dram_tensor("zpad", [128, 4], DT_MM, kind="ExternalInput")
    kw1t_d = nc.dram_tensor("kw1t", [4, 128, 3, DH], DT_MM, kind="ExternalInput")
    kw2t_d = nc.dram_tensor("kw2t", [2, 128, 3, DH], DT_MM, kind="ExternalInput")
    qw1t_d = nc.dram_tensor("qw1t", [D_DEC, 7, DH], DT_MM, kind="ExternalInput")
    qw2t_d = nc.dram_tensor("qw2t", [2, 128, 7, DH], DT_MM, kind="ExternalInput")
    qw3t_d = nc.dram_tensor("qw3t", [2, 128, 7, DH], DT_MM, kind="ExternalInput")
    kb1_d = nc.dram_tensor("kb1c", [2, 128, 1], F32, kind="ExternalInput")
    kb2_d = nc.dram_tensor("kb2c", [2, 128, 1], F32, kind="ExternalInput")
    qb1_d = nc.dram_tensor("qb1c", [2, 128, 1], F32, kind="ExternalInput")
    qb2_d = nc.dram_tensor("qb2c", [2, 128, 1], F32, kind="ExternalInput")
    qb3_d = nc.dram_tensor("qb3c", [2, 128, 1], F32, kind="ExternalInput")
    attn_s = nc.dram_tensor("attn_s", [BPC, TQ, TK2], F32)
    logp_s = nc.dram_tensor("logp_s", [BPC, TQ, TK2], F32)
    canary = nc.dram_tensor("canary", [1, 16], F32, kind="ExternalOutput")

    with tile.TileContext(nc) as tc:
        for _ in range(repeat):
            _emit(nc, tc, q_in, k_in, z_in, kw1t_d, kw2t_d, qw1t_d, qw2t_d,
                  qw3t_d, kb1_d, kb2_d, qb1_d, qb2_d, qb3_d, attn_s, logp_s,
                  mode=mode)
        with tc.tile_pool(name="canary_p", bufs=1) as cp:
            ct = cp.tile([1, 16], F32)
            nc.sync.dma_start(out=ct[:, :], in_=attn_s[0, 0:1, 0:16])
            nc.sync.dma_start(out=canary[:, :], in_=ct[:, :])
    nc.compile()
    return nc


def timing_in_maps(in_maps):
    out = []
    for m in in_maps:
        m2 = dict(m)
        m2["queries"] = m["queries"][0:1]
        m2["keys"] = m["keys"][0:1]
        out.append(m2)
    return out


_PROGRAM = None


def _get_program():
    global _PROGRAM
    if _PROGRAM is None:
        _PROGRAM = build_program()
    return _PROGRAM


def prep_inputs(queries, keys, kw1, kb1, kw2, kb2, qw1, qb1, qw2, qb2, qw3, qb3):
    """Build the 8 per-core input maps from full-size inputs."""
    f = np.float32
    fm = mybir.dt.np(DT_MM)
    kw1t = np.ascontiguousarray(np.transpose(kw1, (1, 2, 0)).reshape(4, 128, 3, DH), fm)
    kw2t = np.ascontiguousarray(np.transpose(kw2, (1, 2, 0)).reshape(2, 128, 3, DH), fm)
    qw1t = np.ascontiguousarray(np.transpose(qw1, (1, 2, 0)), fm)
    qw2t = np.ascontiguousarray(np.transpose(qw2, (1, 2, 0)).reshape(2, 128, 7, DH), fm)
    qw3t = np.ascontiguousarray(np.transpose(qw3, (1, 2, 0)).reshape(2, 128, 7, DH), fm)
    shared = dict(
        kw1t=kw1t, kw2t=kw2t, qw1t=qw1t, qw2t=qw2t, qw3t=qw3t,
        kb1c=np.ascontiguousarray(kb1.reshape(2, 128, 1), f),
        kb2c=np.ascontiguousarray(kb2.reshape(2, 128, 1), f),
        qb1c=np.ascontiguousarray(qb1.reshape(2, 128, 1), f),
        qb2c=np.ascontiguousarray(qb2.reshape(2, 128, 1), f),
        qb3c=np.ascontiguousarray(qb3.reshape(2, 128, 1), f),
    )
    shared["zpad"] = np.zeros((128, 4), fm)
    B = queries.shape[0]
    qp = np.zeros((B, D_DEC, TQ + 6), fm)
    qp[:, :, 3:TQ + 3] = queries
    kp = np.zeros((B, D_ENC, TK + 6), fm)
    kp[:, :, 3:TK + 3] = keys
    in_maps = []
    for i in range(N_CORES):
        m = dict(shared)
        m["queries"] = np.ascontiguousarray(qp[BPC * i:BPC * (i + 1)])
        m["keys"] = np.ascontiguousarray(kp[BPC * i:BPC * (i + 1)])
        in_maps.append(m)
    return in_maps


def run(in_maps, **kwargs):
    nc = _get_program()
    return run_bass_kernel_spmd(nc, in_maps, core_ids=list(range(N_CORES)), **kwargs)


def kernel(queries, keys, kw1, kb1, kw2, kb2, qw1, qb1, qw2, qb2, qw3, qb3,
           **kwargs):
    in_maps = prep_inputs(queries, keys, kw1, kb1, kw2, kb2,
                          qw1, qb1, qw2, qb2, qw3, qb3)
    res = run(in_maps)
    attn = np.concatenate([r["attn_out"] for r in res.results], axis=0)
    logp = np.concatenate([r["logp_out"] for r in res.results], axis=0)
    B = attn.shape[0]
    return attn.reshape(B, 1, TQ, TK2), logp.reshape(B, 1, TQ, TK2)

